# revision 6
# baseline (speedup 1.0000x reference)
"""Angles2BasisDihedral Trainium2 kernel (8 NeuronCores, data-parallel).

Math: per sample b with angles alpha/beta (L=512), per-position rotation
  A_j = Rz(alpha_j) @ Rx(beta_j)  (3x3), cumulative M_p = A_1 @ ... @ A_p,
  output[b, r, 3p+c] = M_p[r][c] for p=0..L (M_0 = I), zeroed for p > len_b.

Device strategy (per core, 2048 samples):
  - lockstep full-chain scan: NL=8 sample lanes per partition scan their full
    chains simultaneously (lanes in the free dim). No chunked scan -> no
    carry phases at all; DVE does only the 42-elem/position recurrence.
  - host pre-wraps angles into [-pi, pi] and ZEROES them past each sample's
    length, so sin()=0 there and A_j becomes the zero matrix: masking costs
    nothing and prefix products past the length are exactly zero.
  - samples are length-sorted and split into G=2 slabs (short/long); each
    slab's scan runs only to the slab's max length.
  - position-strips of SS=64 stream through SBUF double-buffered:
    DMA-in -> ACT(sin, written straight into A^T-tile slots) ->
    GPSIMD(4 remaining A products) -> DVE(scan, writes output layout) ->
    DMA-out. Engine-linear dataflow keeps cross-engine waits minimal.
  - out tile has SS+1 slots; slot 0 carries the previous strip's last M
    (identity for the first strip), so every scan step is uniform and the
    identity column ships with the first strip's DMA.
  - output row mapping s = g*1024 + p*NL + l makes the DRAM (lane, row)
    dims mergeable, keeping every DMA access pattern at <=3 dims.
"""
import math
import os
import numpy as np

ABLATE = os.environ.get("KERNEL_ABLATE", "")
REPS = int(os.environ.get("KERNEL_REPS", "1"))

import concourse.bacc as bacc
import concourse.mybir as mybir
from concourse.bass_utils import run_bass_kernel_spmd
from concourse.tile import TileContext

B, L = 16384, 512
NCORES = 8
BPC = B // NCORES            # samples per core (2048)
NL = 8                       # sample lanes per partition (per slab)
G = 2                        # slabs (length-sorted halves)
SS = 64                      # positions per strip
NSTR = L // SS               # strips per slab in the input layout (8)
SPS = 128 * NL               # samples per slab (1024)
OUTW = 3 * (L + 1)           # 1539 columns per output row
INW = G * NSTR * NL * 4 * SS # input cols per partition row

F32 = mybir.dt.float32
ADD = mybir.AluOpType.add
MULT = mybir.AluOpType.mult

_CACHE = {}


def _build(plans):
    """Build the Bass program. `plans` is a tuple of scan lengths per slab."""
    nc = bacc.Bacc("TRN2", target_bir_lowering=False)
    # const needed for activation scale=-1.0
    t = nc.alloc_sbuf_tensor("const-f32-neg1", [128, 1], F32)
    nc.gpsimd.memset(t.ap(), -1.0)
    nc.const_aps.aps[(F32, -1.0)] = t.ap()
    nc.all_engine_barrier()

    inp = nc.declare_dram_parameter("inp", [128, INW], F32, isOutput=False)
    cst = nc.declare_dram_parameter("cst", [128, 9], F32, isOutput=False)
    out = nc.declare_dram_parameter("out", [BPC, 3 * OUTW], F32, isOutput=True)

    SIN = mybir.ActivationFunctionType.Sin
    IDT = mybir.ActivationFunctionType.Identity

    with TileContext(nc) as tc:
        with (
            tc.tile_pool(name="pcst", bufs=1) as pcst,
            tc.tile_pool(name="pin", bufs=2) as pin,
            tc.tile_pool(name="pns", bufs=2) as pns,
            tc.tile_pool(name="pA", bufs=2) as pA,
            tc.tile_pool(name="pout", bufs=3) as pout,
            tc.tile_pool(name="pwork", bufs=1) as pwork,
        ):
            eye = pcst.tile([128, 9], F32)
            nc.gpsimd.dma_start(out=eye[:, :], in_=cst[:, :])
            # ACT warmup: absorb the const-DMA semaphore into ACT's clock
            warm = pcst.tile([128, 1], F32)
            nc.scalar.activation(warm[:, :], eye[:, 0:1], IDT)
            # zero tile: tails are DMA'd to DRAM from here
            zt = pcst.tile([128, 1536], F32)
            nc.vector.memset(zt[:, :], 0.0)

            for rep in range(REPS):
                for g in range(G):
                    steps = plans[g]
                    # row for (p, l) is g*SPS + p*NL + l -> (l, r) merge to
                    # one stride-1539 dim of 3*NL on the DRAM side
                    odv = (out[g * SPS:(g + 1) * SPS, :]
                           .rearrange("(p l) (r c) -> p l r c",
                                      p=128, l=NL, r=3, c=OUTW))
                    if 3 + 3 * steps < OUTW:
                        tl = OUTW - (3 + 3 * steps)
                        ztv = (zt[:, 0:tl].unsqueeze(1)
                               .broadcast_to([128, NL * 3, tl]))
                        odt = (odv[:, :, :, 3 + 3 * steps:]
                               .rearrange("p l r c -> p (l r) c"))
                        nc.gpsimd.dma_start(out=odt, in_=ztv)
                    if steps == 0 or ABLATE == "dma":
                        continue

                    nstrip = -(-steps // SS)
                    prev_otv = None
                    for st in range(nstrip):
                        j0 = st * SS
                        Sw = min(SS, steps - j0)
                        it = pin.tile([128, NL * 4 * SS], F32, tag="it")
                        off = (g * NSTR + st) * NL * 4 * SS
                        nc.gpsimd.dma_start(out=it[:, :],
                                            in_=inp[:, off:off + NL * 4 * SS])
                        itw = (it[:, :].rearrange("p (l q j) -> p l q j",
                                                  l=NL, q=4, j=SS)
                               [:, :, :, 0:Sw])

                        At = pA.tile([128, NL * SS * 9], F32, tag="At")
                        Atv = At[:, :].rearrange("p (l j e) -> p l j e",
                                                 l=NL, j=SS, e=9)
                        Atw = Atv[:, :, 0:Sw, :]
                        ns = pns.tile([128, NL * 2 * SS], F32, tag="ns")
                        nsw = (ns[:, :].rearrange("p (l x j) -> p l x j",
                                                  l=NL, x=2, j=SS)
                               [:, :, :, 0:Sw])
                        # ACT: sins straight into A^T slots
                        # A^T entry e=c*3+m holds A[m][c]:
                        # [ca, sa, 0, -sa*cb, ca*cb, sb, sa*sb, -ca*sb, cb]
                        sc = nc.scalar
                        sc.activation(Atw[:, :, :, 0], itw[:, :, 1, :], SIN)
                        sc.activation(Atw[:, :, :, 1], itw[:, :, 0, :], SIN)
                        sc.activation(Atw[:, :, :, 5], itw[:, :, 2, :], SIN)
                        sc.activation(Atw[:, :, :, 8], itw[:, :, 3, :], SIN)
                        sc.activation(nsw[:, :, 0, :], itw[:, :, 0, :], SIN,
                                      scale=-1.0)
                        sc.activation(nsw[:, :, 1, :], itw[:, :, 1, :], SIN,
                                      scale=-1.0)
                        # GPSIMD: zero slot + 4 products
                        gp = nc.gpsimd
                        gp.memset(Atw[:, :, :, 2], 0.0)
                        gp.tensor_tensor(out=Atw[:, :, :, 3],
                                         in0=nsw[:, :, 0, :],
                                         in1=Atw[:, :, :, 8], op=MULT)
                        gp.tensor_tensor(out=Atw[:, :, :, 4],
                                         in0=Atw[:, :, :, 0],
                                         in1=Atw[:, :, :, 8], op=MULT)
                        gp.tensor_tensor(out=Atw[:, :, :, 6],
                                         in0=Atw[:, :, :, 1],
                                         in1=Atw[:, :, :, 5], op=MULT)
                        gp.tensor_tensor(out=Atw[:, :, :, 7],
                                         in0=nsw[:, :, 1, :],
                                         in1=Atw[:, :, :, 5], op=MULT)

                        # DVE: lockstep scan; slot k holds M_{j0+k}
                        ot = pout.tile([128, NL * 3 * (SS + 1) * 3], F32,
                                       tag="ot")
                        otv = ot[:, :].rearrange("p (l r j c) -> p l r j c",
                                                 l=NL, r=3, j=SS + 1, c=3)
                        pr = pwork.tile([128, NL * 27], F32, tag="pr")
                        prv = pr[:, :].rearrange("p (l r c m) -> p l r c m",
                                                 l=NL, r=3, c=3, m=3)
                        if ABLATE != "aonly":
                            if st == 0:
                                eye3 = (eye[:, :]
                                        .rearrange("p (r c) -> p r c", r=3, c=3)
                                        .unsqueeze(1)
                                        .broadcast_to([128, NL, 3, 3]))
                                nc.vector.tensor_copy(out=otv[:, :, :, 0, :],
                                                      in_=eye3)
                            else:
                                nc.vector.tensor_copy(
                                    out=otv[:, :, :, 0, :],
                                    in_=prev_otv[:, :, :, SS, :])
                            for k in range(1, Sw + 1):
                                in0 = (otv[:, :, :, k - 1, :].unsqueeze(3)
                                       .broadcast_to([128, NL, 3, 3, 3]))
                                in1 = (Atv[:, :, k - 1, :]
                                       .rearrange("p l (c m) -> p l c m",
                                                  c=3, m=3)
                                       .unsqueeze(2)
                                       .broadcast_to([128, NL, 3, 3, 3]))
                                nc.vector.tensor_tensor(out=prv, in0=in0,
                                                        in1=in1, op=MULT)
                                nc.vector.tensor_tensor(
                                    out=otv[:, :, :, k, :],
                                    in0=prv[:, :, :, :, 0],
                                    in1=prv[:, :, :, :, 1], op=ADD)
                                nc.vector.tensor_tensor(
                                    out=otv[:, :, :, k, 1:3],
                                    in0=otv[:, :, :, k, 1:3],
                                    in1=prv[:, :, :, 1:3, 2], op=ADD)
                        prev_otv = otv
                        k0 = 0 if st == 0 else 1
                        src = (otv[:, :, :, k0:Sw + 1, :]
                               .rearrange("p l r j c -> p (l r) (j c)"))
                        dst = (odv[:, :, :, 3 * (j0 + k0):3 * (j0 + Sw) + 3]
                               .rearrange("p l r c -> p (l r) c"))
                        nc.gpsimd.dma_start(out=dst, in_=src)

    nc.finalize()
    return nc


def _wrap(x):
    return x - (2.0 * np.pi) * np.round(x / (2.0 * np.pi))


def prepare(input, angles_length):
    """Host-side prep: sort/stripe/wrap/mask inputs, build (cached) program."""
    input = np.asarray(input, dtype=np.float32)
    lens = np.asarray(angles_length).astype(np.int64)

    order = np.argsort(lens, kind="stable")
    in_maps = []
    plans = [0] * G
    jj = np.arange(L)[None, :]
    for k in range(NCORES):
        idx = order[k::NCORES]
        lk = lens[idx]
        a = input[idx, 0, :]
        bta = input[idx, 1, :]
        live = (jj < lk[:, None])
        rows = np.stack([
            np.where(live, _wrap(a), 0.0),
            np.where(live, _wrap(a + np.pi / 2), 0.0),
            np.where(live, _wrap(bta), 0.0),
            np.where(live, _wrap(bta + np.pi / 2), 0.0),
        ], axis=1).astype(np.float32)                 # (2048, 4, L)
        # sorted sample s = g*1024 + l*128 + p -> arr[p, (g, strip, l, q, j)]
        arr = (rows.reshape(G, NL, 128, 4, NSTR, SS)
               .transpose(2, 0, 4, 1, 3, 5).reshape(128, INW).copy())
        in_maps.append({"inp": arr})
        for g in range(G):
            plans[g] = max(plans[g], int(lk[(g + 1) * SPS - 1]))

    eye = np.broadcast_to(
        np.eye(3, dtype=np.float32).reshape(9), (128, 9)).copy()
    for m in in_maps:
        m["cst"] = eye

    key = tuple(plans)
    if key not in _CACHE:
        _CACHE[key] = _build(key)
    return _CACHE[key], in_maps, order


# output row s = g*1024 + p*NL + l  <->  sorted sample g*1024 + l*128 + p
_s = np.arange(BPC)
_g, _r = _s // SPS, _s % SPS
_OUT2SORT = _g * SPS + (_r % NL) * 128 + _r // NL


def kernel(input, angles_length):
    nc, in_maps, order = prepare(input, angles_length)
    res = run_bass_kernel_spmd(nc, in_maps, core_ids=list(range(NCORES)))
    full = np.empty((B, 3, OUTW), dtype=np.float32)
    for k in range(NCORES):
        idx = order[k::NCORES]
        full[idx[_OUT2SORT]] = res.results[k]["out"].reshape(BPC, 3, OUTW)
    return full


# revision 7
# speedup vs baseline: 1.1030x; 1.1030x over previous
"""Angles2BasisDihedral Trainium2 kernel (8 NeuronCores, data-parallel).

Math: per sample b with angles alpha/beta (L=512), per-position rotation
  A_j = Rz(alpha_j) @ Rx(beta_j)  (3x3), cumulative M_p = A_1 @ ... @ A_p,
  output[b, r, 3p+c] = M_p[r][c] for p=0..L (M_0 = I), zeroed for p > len_b.

Device strategy (per core, 2048 samples):
  - lockstep full-chain scan: NL=8 sample lanes per partition scan their full
    chains simultaneously (lanes in the free dim). No chunked scan -> no
    carry phases at all; DVE does only the 42-elem/position recurrence.
  - host pre-wraps angles into [-pi, pi] and ZEROES them past each sample's
    length, so sin()=0 there and A_j becomes the zero matrix: masking costs
    nothing and prefix products past the length are exactly zero.
  - samples are length-sorted and split into G=2 slabs (short/long); each
    slab's scan runs only to the slab's max length.
  - position-strips of SS=64 stream through SBUF double-buffered:
    DMA-in -> ACT(sin, written straight into A^T-tile slots) ->
    GPSIMD(4 remaining A products) -> DVE(scan, writes output layout) ->
    DMA-out. Engine-linear dataflow keeps cross-engine waits minimal.
  - out tile has SS+1 slots; slot 0 carries the previous strip's last M
    (identity for the first strip), so every scan step is uniform and the
    identity column ships with the first strip's DMA.
  - output row mapping s = g*1024 + p*NL + l makes the DRAM (lane, row)
    dims mergeable, keeping every DMA access pattern at <=3 dims.
"""
import math
import os
import numpy as np

ABLATE = os.environ.get("KERNEL_ABLATE", "")
REPS = int(os.environ.get("KERNEL_REPS", "1"))

import concourse.bacc as bacc
import concourse.mybir as mybir
from concourse.bass_utils import run_bass_kernel_spmd
from concourse.tile import TileContext

B, L = 16384, 512
NCORES = 8
BPC = B // NCORES            # samples per core (2048)
NL = 8                       # sample lanes per partition (per slab)
G = 2                        # slabs (length-sorted halves)
SS = 64                      # positions per strip
NSTR = L // SS               # strips per slab in the input layout (8)
SPS = 128 * NL               # samples per slab (1024)
OUTW = 3 * (L + 1)           # 1539 columns per output row
INW = G * NSTR * NL * 4 * SS # input cols per partition row

F32 = mybir.dt.float32
ADD = mybir.AluOpType.add
MULT = mybir.AluOpType.mult

_CACHE = {}


def _build(plans):
    """Build the Bass program. `plans` is a tuple of scan lengths per slab."""
    nc = bacc.Bacc("TRN2", target_bir_lowering=False)
    # const needed for activation scale=-1.0
    t = nc.alloc_sbuf_tensor("const-f32-neg1", [128, 1], F32)
    nc.gpsimd.memset(t.ap(), -1.0)
    nc.const_aps.aps[(F32, -1.0)] = t.ap()
    nc.all_engine_barrier()

    inp = nc.declare_dram_parameter("inp", [128, INW], F32, isOutput=False)
    cst = nc.declare_dram_parameter("cst", [128, 9], F32, isOutput=False)
    out = nc.declare_dram_parameter("out", [BPC, 3 * OUTW], F32, isOutput=True)

    SIN = mybir.ActivationFunctionType.Sin
    IDT = mybir.ActivationFunctionType.Identity

    with TileContext(nc) as tc:
        with (
            tc.tile_pool(name="pcst", bufs=1) as pcst,
            tc.tile_pool(name="pin", bufs=2) as pin,
            tc.tile_pool(name="pns", bufs=2) as pns,
            tc.tile_pool(name="pA", bufs=2) as pA,
            tc.tile_pool(name="pout", bufs=3) as pout,
            tc.tile_pool(name="pwork", bufs=1) as pwork,
        ):
            eye = pcst.tile([128, 9], F32)
            nc.gpsimd.dma_start(out=eye[:, :], in_=cst[:, :])
            # ACT warmup: absorb the const-DMA semaphore into ACT's clock
            warm = pcst.tile([128, 1], F32)
            nc.scalar.activation(warm[:, :], eye[:, 0:1], IDT)
            # zero tile: tails are DMA'd to DRAM from here
            zt = pcst.tile([128, 1536], F32)
            nc.vector.memset(zt[:, :], 0.0)

            def odv_for(g):
                # row for (p, l) is g*SPS + p*NL + l -> (l, r) merge to
                # one stride-1539 dim of 3*NL on the DRAM side
                return (out[g * SPS:(g + 1) * SPS, :]
                        .rearrange("(p l) (r c) -> p l r c",
                                   p=128, l=NL, r=3, c=OUTW))

            def emit_tail(g, steps):
                if 3 + 3 * steps < OUTW:
                    tl = OUTW - (3 + 3 * steps)
                    ztv = (zt[:, 0:tl].unsqueeze(1)
                           .broadcast_to([128, NL * 3, tl]))
                    odt = (odv_for(g)[:, :, :, 3 + 3 * steps:]
                           .rearrange("p l r c -> p (l r) c"))
                    nc.gpsimd.dma_start(out=odt, in_=ztv)

            def emit_feed(stage):
                """DMA-in + ACT sins + GPSIMD A-products for one strip."""
                g, st, Sw = stage
                it = pin.tile([128, NL * 4 * SS], F32, tag="it")
                off = (g * NSTR + st) * NL * 4 * SS
                nc.gpsimd.dma_start(out=it[:, :],
                                    in_=inp[:, off:off + NL * 4 * SS])
                itw = (it[:, :].rearrange("p (l q j) -> p l q j",
                                          l=NL, q=4, j=SS)[:, :, :, 0:Sw])
                At = pA.tile([128, NL * SS * 9], F32, tag="At")
                Atv = At[:, :].rearrange("p (l j e) -> p l j e",
                                         l=NL, j=SS, e=9)
                Atw = Atv[:, :, 0:Sw, :]
                ns = pns.tile([128, NL * 2 * SS], F32, tag="ns")
                nsw = (ns[:, :].rearrange("p (l x j) -> p l x j",
                                          l=NL, x=2, j=SS)[:, :, :, 0:Sw])
                # ACT: sins straight into A^T slots
                # A^T entry e=c*3+m holds A[m][c]:
                # [ca, sa, 0, -sa*cb, ca*cb, sb, sa*sb, -ca*sb, cb]
                sc = nc.scalar
                sc.activation(Atw[:, :, :, 0], itw[:, :, 1, :], SIN)
                sc.activation(Atw[:, :, :, 1], itw[:, :, 0, :], SIN)
                sc.activation(Atw[:, :, :, 5], itw[:, :, 2, :], SIN)
                sc.activation(Atw[:, :, :, 8], itw[:, :, 3, :], SIN)
                sc.activation(nsw[:, :, 0, :], itw[:, :, 0, :], SIN,
                              scale=-1.0)
                sc.activation(nsw[:, :, 1, :], itw[:, :, 1, :], SIN,
                              scale=-1.0)
                # GPSIMD: zero slot + 4 products
                gp = nc.gpsimd
                gp.memset(Atw[:, :, :, 2], 0.0)
                gp.tensor_tensor(out=Atw[:, :, :, 3], in0=nsw[:, :, 0, :],
                                 in1=Atw[:, :, :, 8], op=MULT)
                gp.tensor_tensor(out=Atw[:, :, :, 4], in0=Atw[:, :, :, 0],
                                 in1=Atw[:, :, :, 8], op=MULT)
                gp.tensor_tensor(out=Atw[:, :, :, 6], in0=Atw[:, :, :, 1],
                                 in1=Atw[:, :, :, 5], op=MULT)
                gp.tensor_tensor(out=Atw[:, :, :, 7], in0=nsw[:, :, 1, :],
                                 in1=Atw[:, :, :, 5], op=MULT)
                return Atv

            def emit_scan(stage, Atv, prev_otv):
                """DVE lockstep scan + DMA-out; slot k holds M_{j0+k}."""
                g, st, Sw = stage
                j0 = st * SS
                ot = pout.tile([128, NL * 3 * (SS + 1) * 3], F32, tag="ot")
                otv = ot[:, :].rearrange("p (l r j c) -> p l r j c",
                                         l=NL, r=3, j=SS + 1, c=3)
                pr = pwork.tile([128, NL * 27], F32, tag="pr")
                prv = pr[:, :].rearrange("p (l r c m) -> p l r c m",
                                         l=NL, r=3, c=3, m=3)
                if ABLATE != "aonly":
                    if st == 0:
                        eye3 = (eye[:, :]
                                .rearrange("p (r c) -> p r c", r=3, c=3)
                                .unsqueeze(1).broadcast_to([128, NL, 3, 3]))
                        nc.vector.tensor_copy(out=otv[:, :, :, 0, :], in_=eye3)
                    else:
                        nc.vector.tensor_copy(out=otv[:, :, :, 0, :],
                                              in_=prev_otv[:, :, :, SS, :])
                    for k in range(1, Sw + 1):
                        in0 = (otv[:, :, :, k - 1, :].unsqueeze(3)
                               .broadcast_to([128, NL, 3, 3, 3]))
                        in1 = (Atv[:, :, k - 1, :]
                               .rearrange("p l (c m) -> p l c m", c=3, m=3)
                               .unsqueeze(2)
                               .broadcast_to([128, NL, 3, 3, 3]))
                        nc.vector.tensor_tensor(out=prv, in0=in0, in1=in1,
                                                op=MULT)
                        nc.vector.tensor_tensor(out=otv[:, :, :, k, :],
                                                in0=prv[:, :, :, :, 0],
                                                in1=prv[:, :, :, :, 1],
                                                op=ADD)
                        nc.vector.tensor_tensor(out=otv[:, :, :, k, 1:3],
                                                in0=otv[:, :, :, k, 1:3],
                                                in1=prv[:, :, :, 1:3, 2],
                                                op=ADD)
                k0 = 0 if st == 0 else 1
                src = (otv[:, :, :, k0:Sw + 1, :]
                       .rearrange("p l r j c -> p (l r) (j c)"))
                dst = (odv_for(g)[:, :, :, 3 * (j0 + k0):3 * (j0 + Sw) + 3]
                       .rearrange("p l r c -> p (l r) c"))
                nc.gpsimd.dma_start(out=dst, in_=src)
                return otv

            # flat strip list across reps/slabs, software-pipelined by one:
            # feed(i+1) is emitted before scan(i)'s blocking DMA-out so
            # GPSIMD/ACT prepare the next strip while DVE scans this one.
            stages = []
            for rep in range(REPS):
                for g in range(G):
                    steps = plans[g]
                    emit_tail(g, steps)
                    if steps == 0 or ABLATE == "dma":
                        continue
                    for st in range(-(-steps // SS)):
                        stages.append((g, st, min(SS, steps - st * SS)))
            Atv_next = emit_feed(stages[0]) if stages else None
            prev_otv = None
            for i, stage in enumerate(stages):
                Atv = Atv_next
                if i + 1 < len(stages):
                    Atv_next = emit_feed(stages[i + 1])
                prev_otv = emit_scan(stage, Atv, prev_otv)

    nc.finalize()
    return nc


def _wrap(x):
    return x - (2.0 * np.pi) * np.round(x / (2.0 * np.pi))


def prepare(input, angles_length):
    """Host-side prep: sort/stripe/wrap/mask inputs, build (cached) program."""
    input = np.asarray(input, dtype=np.float32)
    lens = np.asarray(angles_length).astype(np.int64)

    order = np.argsort(lens, kind="stable")
    in_maps = []
    plans = [0] * G
    jj = np.arange(L)[None, :]
    for k in range(NCORES):
        idx = order[k::NCORES]
        lk = lens[idx]
        a = input[idx, 0, :]
        bta = input[idx, 1, :]
        live = (jj < lk[:, None])
        rows = np.stack([
            np.where(live, _wrap(a), 0.0),
            np.where(live, _wrap(a + np.pi / 2), 0.0),
            np.where(live, _wrap(bta), 0.0),
            np.where(live, _wrap(bta + np.pi / 2), 0.0),
        ], axis=1).astype(np.float32)                 # (2048, 4, L)
        # sorted sample s = g*1024 + l*128 + p -> arr[p, (g, strip, l, q, j)]
        arr = (rows.reshape(G, NL, 128, 4, NSTR, SS)
               .transpose(2, 0, 4, 1, 3, 5).reshape(128, INW).copy())
        in_maps.append({"inp": arr})
        for g in range(G):
            plans[g] = max(plans[g], int(lk[(g + 1) * SPS - 1]))

    eye = np.broadcast_to(
        np.eye(3, dtype=np.float32).reshape(9), (128, 9)).copy()
    for m in in_maps:
        m["cst"] = eye

    key = tuple(plans)
    if key not in _CACHE:
        _CACHE[key] = _build(key)
    return _CACHE[key], in_maps, order


# output row s = g*1024 + p*NL + l  <->  sorted sample g*1024 + l*128 + p
_s = np.arange(BPC)
_g, _r = _s // SPS, _s % SPS
_OUT2SORT = _g * SPS + (_r % NL) * 128 + _r // NL


def kernel(input, angles_length):
    nc, in_maps, order = prepare(input, angles_length)
    res = run_bass_kernel_spmd(nc, in_maps, core_ids=list(range(NCORES)))
    full = np.empty((B, 3, OUTW), dtype=np.float32)
    for k in range(NCORES):
        idx = order[k::NCORES]
        full[idx[_OUT2SORT]] = res.results[k]["out"].reshape(BPC, 3, OUTW)
    return full


# revision 9
# speedup vs baseline: 1.5271x; 1.3844x over previous
"""Angles2BasisDihedral Trainium2 kernel (8 NeuronCores, data-parallel).

Math: per sample b with angles alpha/beta (L=512), per-position rotation
  A_j = Rz(alpha_j) @ Rx(beta_j)  (3x3), cumulative M_p = A_1 @ ... @ A_p,
  output[b, r, 3p+c] = M_p[r][c] for p=0..L (M_0 = I), zeroed for p > len_b.

Device strategy (per core, 2048 samples):
  - lockstep full-chain scan: NL=8 sample lanes per partition scan their full
    chains simultaneously (lanes in the free dim). No chunked scan -> no
    carry phases; DVE does only the per-position recurrence (mult + reduce).
  - host pre-wraps angles into [-pi, pi] and ZEROES them past each sample's
    length, so sin()=0 there and A_j becomes the zero matrix: masking costs
    nothing and prefix products past the length are exactly zero.
  - samples are length-sorted into G=2 slabs (short/long); each slab's scan
    runs only to the slab's max length.
  - TWO WORKERS, one per slab, with their scan steps interleaved on DVE:
    adjacent instructions then touch different tiles, which hides the
    engine's memory-conflict stall (measured 879 -> 602 ns/step on HW).
    After the short slab finishes, the long slab runs solo with its pr
    buffer rotated to keep write-after-read conflicts spaced out.
  - position-strips of SS=48 stream through SBUF double-buffered per worker:
    DMA-in -> ACT(sin, written straight into A^T slots) -> GPSIMD(4 products)
    -> DVE(scan, writes output layout) -> DMA-out, feeds prefetched one
    strip ahead.
  - out tile has SS+1 slots; slot 0 carries the previous strip's last M
    (identity for the first strip), so the identity column ships with the
    first strip's DMA.
  - output row mapping s = g*1024 + p*NL + l keeps every DMA access pattern
    at <=3 dims (the DRAM (lane, row) dims merge).
"""
import math
import os
import numpy as np

ABLATE = os.environ.get("KERNEL_ABLATE", "")
REPS = int(os.environ.get("KERNEL_REPS", "1"))

import concourse.bacc as bacc
import concourse.mybir as mybir
from concourse.bass_utils import run_bass_kernel_spmd
from concourse.tile import TileContext

B, L = 16384, 512
NCORES = 8
BPC = B // NCORES            # samples per core (2048)
NL = 8                       # sample lanes per partition (per slab)
G = 2                        # slabs (length-sorted halves)
SS = 48                      # positions per strip
NSTR = -(-L // SS)           # strips per slab in the input layout (11)
LPAD = NSTR * SS             # padded chain length (528)
SPS = 128 * NL               # samples per slab (1024)
OUTW = 3 * (L + 1)           # 1539 columns per output row
INW = G * NSTR * NL * 4 * SS # input cols per partition row

F32 = mybir.dt.float32
ADD = mybir.AluOpType.add
MULT = mybir.AluOpType.mult

_CACHE = {}


class _Worker:
    """Per-slab scan pipeline: strips of SS positions, feeds prefetched one
    strip ahead; step() emits one DVE scan step (or boundary copy)."""

    def __init__(self, nc, tc, pools, wid, g, steps, inp, odv, eye):
        self.nc = nc
        self.wid = wid
        self.g = g
        self.steps = steps
        self.inp = inp
        self.odv = odv
        self.eye = eye
        self.pin, self.pns, self.pA, self.pout, self.pw = pools
        self.nstrip = -(-steps // SS) if steps > 0 else 0
        self.st = 0              # current strip
        self.k = 0               # next slot to write (1..Sw); 0 = slot0 copy
        self.otv = None
        self.prev_otv = None
        self.Atv = None
        self.Atv_next = None
        self.prs = []
        for q in range(3):
            pr = self.pw.tile([128, NL * 27], F32, tag=f"pr{wid}_{q}")
            self.prs.append(pr[:, :].rearrange("p (l r c m) -> p l r c m",
                                               l=NL, r=3, c=3, m=3))
        self.nk = 0              # rotation counter
        if self.nstrip:
            self.Atv_next = self._feed(0)

    def _sw(self, st):
        return min(SS, self.steps - st * SS)

    def _feed(self, st):
        """DMA-in + ACT sins + GPSIMD A-products for strip st."""
        nc = self.nc
        Sw = self._sw(st)
        it = self.pin.tile([128, NL * 4 * SS], F32, tag=f"it{self.wid}")
        off = (self.g * NSTR + st) * NL * 4 * SS
        nc.gpsimd.dma_start(out=it[:, :],
                            in_=self.inp[:, off:off + NL * 4 * SS])
        itw = (it[:, :].rearrange("p (l q j) -> p l q j",
                                  l=NL, q=4, j=SS)[:, :, :, 0:Sw])
        At = self.pA.tile([128, NL * SS * 9], F32, tag=f"At{self.wid}")
        Atv = At[:, :].rearrange("p (l j e) -> p l j e", l=NL, j=SS, e=9)
        Atw = Atv[:, :, 0:Sw, :]
        ns = self.pns.tile([128, NL * 2 * SS], F32, tag=f"ns{self.wid}")
        nsw = (ns[:, :].rearrange("p (l x j) -> p l x j",
                                  l=NL, x=2, j=SS)[:, :, :, 0:Sw])
        # ACT: sins straight into A^T slots; A^T entry e=c*3+m holds A[m][c]:
        # [ca, sa, 0, -sa*cb, ca*cb, sb, sa*sb, -ca*sb, cb]
        SIN = mybir.ActivationFunctionType.Sin
        sc = nc.scalar
        sc.activation(Atw[:, :, :, 0], itw[:, :, 1, :], SIN)
        sc.activation(Atw[:, :, :, 1], itw[:, :, 0, :], SIN)
        sc.activation(Atw[:, :, :, 5], itw[:, :, 2, :], SIN)
        sc.activation(Atw[:, :, :, 8], itw[:, :, 3, :], SIN)
        sc.activation(nsw[:, :, 0, :], itw[:, :, 0, :], SIN, scale=-1.0)
        sc.activation(nsw[:, :, 1, :], itw[:, :, 1, :], SIN, scale=-1.0)
        # GPSIMD: zero slot + 4 products
        gp = nc.gpsimd
        gp.memset(Atw[:, :, :, 2], 0.0)
        gp.tensor_tensor(out=Atw[:, :, :, 3], in0=nsw[:, :, 0, :],
                         in1=Atw[:, :, :, 8], op=MULT)
        gp.tensor_tensor(out=Atw[:, :, :, 4], in0=Atw[:, :, :, 0],
                         in1=Atw[:, :, :, 8], op=MULT)
        gp.tensor_tensor(out=Atw[:, :, :, 6], in0=Atw[:, :, :, 1],
                         in1=Atw[:, :, :, 5], op=MULT)
        gp.tensor_tensor(out=Atw[:, :, :, 7], in0=nsw[:, :, 1, :],
                         in1=Atw[:, :, :, 5], op=MULT)
        return Atv

    def _flush(self, st):
        """DMA strip st's slots out to DRAM."""
        Sw = self._sw(st)
        j0 = st * SS
        k0 = 0 if st == 0 else 1
        src = (self.otv[:, :, :, k0:Sw + 1, :]
               .rearrange("p l r j c -> p (l r) (j c)"))
        dst = (self.odv[:, :, :, 3 * (j0 + k0):3 * (j0 + Sw) + 3]
               .rearrange("p l r c -> p (l r) c"))
        self.nc.gpsimd.dma_start(out=dst, in_=src)

    @property
    def active(self):
        return self.st < self.nstrip

    def step(self):
        """Emit the next DVE unit: slot0 copy at strip start, else one
        (mult + reduce) scan step. Handles feeds/flushes at boundaries."""
        nc = self.nc
        Sw = self._sw(self.st)
        if self.k == 0:
            self.Atv = self.Atv_next
            self.Atv_next = None
            ot = self.pout.tile([128, NL * 3 * (SS + 1) * 3], F32,
                                tag=f"ot{self.wid}")
            self.otv = ot[:, :].rearrange("p (l r j c) -> p l r j c",
                                          l=NL, r=3, j=SS + 1, c=3)
            if self.st == 0:
                eye3 = (self.eye[:, :]
                        .rearrange("p (r c) -> p r c", r=3, c=3)
                        .unsqueeze(1).broadcast_to([128, NL, 3, 3]))
                nc.vector.tensor_copy(out=self.otv[:, :, :, 0, :], in_=eye3)
            else:
                nc.vector.tensor_copy(out=self.otv[:, :, :, 0, :],
                                      in_=self.prev_otv[:, :, :, SS, :])
            self.k = 1
            return
        if self.k == 2 and self.st + 1 < self.nstrip:
            self.Atv_next = self._feed(self.st + 1)  # prefetch next strip
        k = self.k
        prv = self.prs[self.nk % 3]
        self.nk += 1
        in0 = (self.otv[:, :, :, k - 1, :].unsqueeze(3)
               .broadcast_to([128, NL, 3, 3, 3]))
        in1 = (self.Atv[:, :, k - 1, :]
               .rearrange("p l (c m) -> p l c m", c=3, m=3)
               .unsqueeze(2).broadcast_to([128, NL, 3, 3, 3]))
        nc.vector.tensor_tensor(out=prv, in0=in0, in1=in1, op=MULT)
        nc.vector.tensor_reduce(out=self.otv[:, :, :, k, :], in_=prv,
                                axis=mybir.AxisListType.X, op=ADD)
        if k == Sw:
            self._flush(self.st)
            self.prev_otv = self.otv
            self.st += 1
            self.k = 0
        else:
            self.k = k + 1


def _build(plans):
    """Build the Bass program. `plans` is a tuple of scan lengths per slab."""
    nc = bacc.Bacc("TRN2", target_bir_lowering=False)
    # const needed for activation scale=-1.0
    t = nc.alloc_sbuf_tensor("const-f32-neg1", [128, 1], F32)
    nc.gpsimd.memset(t.ap(), -1.0)
    nc.const_aps.aps[(F32, -1.0)] = t.ap()
    nc.all_engine_barrier()

    inp = nc.declare_dram_parameter("inp", [128, INW], F32, isOutput=False)
    cst = nc.declare_dram_parameter("cst", [128, 9], F32, isOutput=False)
    out = nc.declare_dram_parameter("out", [BPC, 3 * OUTW], F32, isOutput=True)

    IDT = mybir.ActivationFunctionType.Identity

    with TileContext(nc) as tc:
        with (
            tc.tile_pool(name="pcst", bufs=1) as pcst,
            tc.tile_pool(name="pin0", bufs=2) as pin0,
            tc.tile_pool(name="pns0", bufs=2) as pns0,
            tc.tile_pool(name="pA0", bufs=2) as pA0,
            tc.tile_pool(name="pout0", bufs=3) as pout0,
            tc.tile_pool(name="pw0", bufs=3) as pw0,
            tc.tile_pool(name="pin1", bufs=2) as pin1,
            tc.tile_pool(name="pns1", bufs=2) as pns1,
            tc.tile_pool(name="pA1", bufs=2) as pA1,
            tc.tile_pool(name="pout1", bufs=3) as pout1,
            tc.tile_pool(name="pw1", bufs=3) as pw1,
        ):
            eye = pcst.tile([128, 9], F32)
            nc.gpsimd.dma_start(out=eye[:, :], in_=cst[:, :])
            # ACT warmup: absorb the const-DMA semaphore into ACT's clock
            warm = pcst.tile([128, 1], F32)
            nc.scalar.activation(warm[:, :], eye[:, 0:1], IDT)
            # zero tile: tails are DMA'd to DRAM from here
            zt = pcst.tile([128, 1536], F32)
            nc.vector.memset(zt[:, :], 0.0)

            pools = [(pin0, pns0, pA0, pout0, pw0),
                     (pin1, pns1, pA1, pout1, pw1)]

            def odv_for(g):
                return (out[g * SPS:(g + 1) * SPS, :]
                        .rearrange("(p l) (r c) -> p l r c",
                                   p=128, l=NL, r=3, c=OUTW))

            for rep in range(REPS):
                for g in range(G):
                    steps = plans[g]
                    if 3 + 3 * steps < OUTW:
                        tl = OUTW - (3 + 3 * steps)
                        ztv = (zt[:, 0:tl].unsqueeze(1)
                               .broadcast_to([128, NL * 3, tl]))
                        odt = (odv_for(g)[:, :, :, 3 + 3 * steps:]
                               .rearrange("p l r c -> p (l r) c"))
                        nc.gpsimd.dma_start(out=odt, in_=ztv)
                if ABLATE == "dma":
                    continue
                workers = [
                    _Worker(nc, tc, pools[g], str(g), g, plans[g],
                            inp, odv_for(g), eye)
                    for g in range(G)
                ]
                # interleave the workers' scan steps on DVE
                while any(w.active for w in workers):
                    for w in workers:
                        if w.active:
                            w.step()

    nc.finalize()
    return nc


def _wrap(x):
    return x - (2.0 * np.pi) * np.round(x / (2.0 * np.pi))


def prepare(input, angles_length):
    """Host-side prep: sort/stripe/wrap/mask inputs, build (cached) program."""
    input = np.asarray(input, dtype=np.float32)
    lens = np.asarray(angles_length).astype(np.int64)

    order = np.argsort(lens, kind="stable")
    in_maps = []
    plans = [0] * G
    jj = np.arange(L)[None, :]
    for k in range(NCORES):
        idx = order[k::NCORES]
        lk = lens[idx]
        a = input[idx, 0, :]
        bta = input[idx, 1, :]
        live = (jj < lk[:, None])
        rows = np.stack([
            np.where(live, _wrap(a), 0.0),
            np.where(live, _wrap(a + np.pi / 2), 0.0),
            np.where(live, _wrap(bta), 0.0),
            np.where(live, _wrap(bta + np.pi / 2), 0.0),
        ], axis=1).astype(np.float32)                 # (2048, 4, L)
        rows = np.concatenate(
            [rows, np.zeros((BPC, 4, LPAD - L), np.float32)], axis=2)
        # sorted sample s = g*1024 + l*128 + p -> arr[p, (g, strip, l, q, j)]
        arr = (rows.reshape(G, NL, 128, 4, NSTR, SS)
               .transpose(2, 0, 4, 1, 3, 5).reshape(128, INW).copy())
        in_maps.append({"inp": arr})
        for g in range(G):
            plans[g] = max(plans[g], int(lk[(g + 1) * SPS - 1]))

    eye = np.broadcast_to(
        np.eye(3, dtype=np.float32).reshape(9), (128, 9)).copy()
    for m in in_maps:
        m["cst"] = eye

    key = tuple(plans)
    if key not in _CACHE:
        _CACHE[key] = _build(key)
    return _CACHE[key], in_maps, order


# output row s = g*1024 + p*NL + l  <->  sorted sample g*1024 + l*128 + p
_s = np.arange(BPC)
_g, _r = _s // SPS, _s % SPS
_OUT2SORT = _g * SPS + (_r % NL) * 128 + _r // NL


def kernel(input, angles_length):
    nc, in_maps, order = prepare(input, angles_length)
    res = run_bass_kernel_spmd(nc, in_maps, core_ids=list(range(NCORES)))
    full = np.empty((B, 3, OUTW), dtype=np.float32)
    for k in range(NCORES):
        idx = order[k::NCORES]
        full[idx[_OUT2SORT]] = res.results[k]["out"].reshape(BPC, 3, OUTW)
    return full
